# revision 1
# baseline (speedup 1.0000x reference)
"""LoRA layer kernel for Trainium2, 8-core data-parallel.

out = x @ W.T + 2.0 * ((x @ B) @ A)
  x: (4, 4096, 4096) f32, W: (4096, 4096), A: (16, 4096), B: (4096, 16)

Strategy: flatten x to (16384, 4096) rows, shard rows across 8 cores
(2048 rows each), replicate W/A/B. Per core a single fused GEMM:
  - x-block stationary (fp32r), W.T streamed as moving operand
  - LoRA: tT = (x @ B).T computed per block (contraction over full K),
    then one extra K=16 matmul per (m_tile, o_chunk) accumulates
    2*(x@B)@A into the same PSUM bank (A pre-scaled by 2 on host).
All device matmuls use float32r: 1 cycle/row at N=512 (same rate as
bf16, ~TF32 precision).
"""

import sys

if "/opt/trn_rl_repo" not in sys.path:
    sys.path.insert(0, "/opt/trn_rl_repo")

import os

import numpy as np

import concourse.bass as bass
import concourse.mybir as mybir
import concourse.tile as tile

N_CORES = 8
D = 4096
RANK = 16
ROWS_TOTAL = 4 * 4096          # 16384
ROWS_PER_CORE = ROWS_TOTAL // N_CORES  # 2048
P = 128
KT = D // P                    # 32 k-tiles
M_BLOCK = 1024                 # rows per x-resident block
N_BLOCKS = ROWS_PER_CORE // M_BLOCK    # 2
MT_PER_BLOCK = M_BLOCK // P    # 8 m-tiles (PSUM banks)
OC = 512                       # o-chunk width (one PSUM bank)
N_OC = D // OC                 # 8
KH = KT // 2                   # k-tiles per x half-tile

F32 = mybir.dt.float32
F32R = mybir.dt.float32r

W_PAIR = os.environ.get("K_WPAIR", "1") == "1"
GP_DMA = os.environ.get("K_GPDMA", "1") == "1"
WARMUP = os.environ.get("K_WARMUP", "1") == "1"


def _dma_gp(nc):
    return nc.gpsimd if GP_DMA else nc.sync


def split_wide_waits(nc, max_waits=1):
    """walrus in this container rejects >1 sync wait per instruction;
    move excess waits onto preceding same-engine NoOps."""
    n_split = 0
    for f in nc.m.functions:
        for bb in f.blocks:
            new_insts = []
            for inst in bb.instructions:
                si = getattr(inst, "sync_info", None)
                if si is not None and si.on_wait and len(si.on_wait) > max_waits:
                    waits = list(si.on_wait)
                    keep = waits[-max_waits:]
                    extra = waits[:-max_waits]
                    for i in range(0, len(extra), max_waits):
                        chunk = extra[i:i + max_waits]
                        nop = mybir.InstNoOp(
                            name=f"{inst.name}_wsplit{i}",
                            sync_info=mybir.SyncInfo(on_wait=chunk, on_update=[]),
                            bass_nofuse=True,
                            engine=inst.engine,
                        )
                        new_insts.append(nop)
                        n_split += 1
                    si.on_wait = keep
                new_insts.append(inst)
            bb.instructions[:] = new_insts
    return n_split


def build_program():
    nc = bass.Bass()
    xt = nc.declare_dram_parameter("xt", [D, ROWS_PER_CORE], F32R, isOutput=False)
    wt = nc.declare_dram_parameter("wt", [D, D], F32R, isOutput=False)
    # bmat pre-arranged on host: [128, KT*RANK], col-block k = rows k*128..+128
    bmat = nc.declare_dram_parameter("bmat", [P, KT * RANK], F32R, isOutput=False)
    a2 = nc.declare_dram_parameter("a2", [RANK, D], F32R, isOutput=False)
    out = nc.declare_dram_parameter("out", [ROWS_PER_CORE, D], F32, isOutput=True)

    with tile.TileContext(nc) as tc:
        with (
            tc.tile_pool(name="xpool_a", bufs=1) as xpool_a,
            tc.tile_pool(name="xpool_b", bufs=1) as xpool_b,
            tc.tile_pool(name="wpool", bufs=6) as wpool,
            tc.tile_pool(name="opool", bufs=4) as opool,
            tc.tile_pool(name="cpool", bufs=1) as cpool,
            tc.tile_pool(name="tpool", bufs=2) as tpool,
            tc.tile_pool(name="ppool", bufs=8, space="PSUM") as ppool,
        ):
            # constants: B (pre-arranged) and A2 — single DMAs on gpsimd queue
            btile = cpool.tile([P, KT * RANK], F32R, tag="bt")
            _dma_gp(nc).dma_start(btile[:], bmat[:])
            atile = cpool.tile([RANK, D], F32R, tag="at")
            _dma_gp(nc).dma_start(atile[:], a2[:])

            # HAM warmup: ~5us of dummy matmuls so the PE clock is at 8/8
            # before real work lands (3.4us busy window un-throttles).
            if WARMUP:
                junk = ppool.tile([RANK, OC], F32, tag="acc", name="junk")
                for i in range(25):
                    nc.tensor.matmul(
                        junk[:],
                        btile[:, :RANK],
                        btile[:, :OC],
                        start=(i == 0),
                        stop=(i == 24),
                    )

            for blk in range(N_BLOCKS):
                r0 = blk * M_BLOCK
                # x block resident: two half tiles (k 0-15, k 16-31)
                xa = xpool_a.tile([P, KH * M_BLOCK], F32R, tag="xa")
                xb = xpool_b.tile([P, KH * M_BLOCK], F32R, tag="xb")

                def xsl(k, c0, cw):
                    t = xa if k < KH else xb
                    kk = k % KH
                    return t[:, kk * M_BLOCK + c0: kk * M_BLOCK + c0 + cw]

                for k in range(KT):
                    eng = (nc.gpsimd if k % 2 == 0 else nc.scalar) if GP_DMA else nc.sync
                    eng.dma_start(
                        xsl(k, 0, M_BLOCK),
                        xt[k * P:(k + 1) * P, r0:r0 + M_BLOCK],
                    )

                # stage A: tT[r, m] = sum_i B[i,r] * x[m,i]  (per block)
                tT = tpool.tile([RANK, M_BLOCK], F32R, tag="tT")
                for h in range(M_BLOCK // OC):
                    pt = ppool.tile([RANK, OC], F32, tag="acc")
                    for k in range(KT):
                        nc.tensor.matmul(
                            pt[:],
                            btile[:, k * RANK:(k + 1) * RANK],
                            xsl(k, h * OC, OC),
                            start=(k == 0),
                            stop=(k == KT - 1),
                        )
                    nc.vector.tensor_copy(tT[:, h * OC:(h + 1) * OC], pt[:])

                # main GEMM + fused LoRA accumulation.
                # W fetched as adjacent k-tile pairs [128, 2*OC] (halves the
                # ~0.6us/DMA issue count on the sync queue).
                for oc in range(N_OC):
                    psums = []
                    for mt in range(MT_PER_BLOCK):
                        psums.append(ppool.tile([P, OC], F32, tag="acc", name=f"ps_{blk}_{oc}_{mt}"))
                    for k2 in range(KT // 2):
                        wtile = wpool.tile([P, 2 * OC], F32R, tag="wt")
                        src = wt[k2 * 2 * P:(k2 + 1) * 2 * P,
                                 oc * OC:(oc + 1) * OC]
                        if W_PAIR:
                            nc.sync.dma_start(
                                wtile.rearrange("p (b c) -> p b c", b=2),
                                src.rearrange("(b p) c -> p b c", p=P),
                            )
                        else:
                            for half in range(2):
                                nc.sync.dma_start(
                                    wtile[:, half * OC:(half + 1) * OC],
                                    wt[(2 * k2 + half) * P:(2 * k2 + half + 1) * P,
                                       oc * OC:(oc + 1) * OC],
                                )
                        for half in range(2):
                            k = 2 * k2 + half
                            for mt in range(MT_PER_BLOCK):
                                nc.tensor.matmul(
                                    psums[mt][:],
                                    xsl(k, mt * P, P),
                                    wtile[:, half * OC:(half + 1) * OC],
                                    start=(k == 0),
                                    stop=False,
                                )
                    for mt in range(MT_PER_BLOCK):
                        # LoRA: += tT[:, mt].T @ (2A[:, oc])
                        nc.tensor.matmul(
                            psums[mt][:],
                            tT[:, mt * P:(mt + 1) * P],
                            atile[:, oc * OC:(oc + 1) * OC],
                            start=False,
                            stop=True,
                        )
                        ot = opool.tile([P, OC], F32, tag="ot")
                        nc.vector.tensor_copy(ot[:], psums[mt][:])
                        nc.sync.dma_start(
                            out[r0 + mt * P:r0 + (mt + 1) * P,
                                oc * OC:(oc + 1) * OC],
                            ot[:],
                        )

    split_wide_waits(nc)
    return nc


_NC_CACHE = [None]


def kernel(x, weight, lora_A, lora_B):
    from concourse.bass_utils import run_bass_kernel_spmd

    x = np.asarray(x, dtype=np.float32)
    weight = np.asarray(weight, dtype=np.float32)
    lora_A = np.asarray(lora_A, dtype=np.float32)
    lora_B = np.asarray(lora_B, dtype=np.float32)

    x2 = x.reshape(ROWS_TOTAL, D)
    wt = np.ascontiguousarray(weight.T)
    a2 = np.ascontiguousarray(2.0 * lora_A)
    # pre-arrange B: [128, KT*RANK], col-block k holds rows k*128..(k+1)*128
    bmat = np.ascontiguousarray(
        lora_B.reshape(KT, P, RANK).transpose(1, 0, 2).reshape(P, KT * RANK)
    )

    in_maps = []
    for c in range(N_CORES):
        xt_c = np.ascontiguousarray(
            x2[c * ROWS_PER_CORE:(c + 1) * ROWS_PER_CORE].T
        )
        in_maps.append({"xt": xt_c, "wt": wt, "bmat": bmat, "a2": a2})

    if _NC_CACHE[0] is None:
        _NC_CACHE[0] = build_program()
    nc = _NC_CACHE[0]

    res = run_bass_kernel_spmd(nc, in_maps, list(range(N_CORES)))
    out = np.concatenate(
        [res.results[c]["out"] for c in range(N_CORES)], axis=0
    )
    return out.reshape(x.shape)



# revision 4
# speedup vs baseline: 1.0620x; 1.0620x over previous
"""LoRA layer kernel for Trainium2, 8-core data-parallel.

out = x @ W.T + 2.0 * ((x @ B) @ A)
  x: (4, 4096, 4096) f32, W: (4096, 4096), A: (16, 4096), B: (4096, 16)

Strategy: flatten x to (16384, 4096) rows, shard rows across 8 cores
(2048 rows each), replicate W/A/B. All matmul operands in bf16 (PSUM
accumulation stays fp32; end-to-end rel err ~2.4e-3 vs the 2e-2 gate).

Per core, single x-resident block (2048 rows = 128 KB/partition bf16):
  - stage A: tT = (x @ B).T computed 4-way column-tiled: k-tile k goes
    to PE col group k%4 (derived from out base partition 32j), partials
    land in 4 PSUM banks at partition stripes {32j..32j+15}; copies
    assemble tT4 [128, 2048] with zeros (memset) in the gap stripes.
  - main GEMM: per o-chunk (512 cols), 4 mt-quarters of 4 PSUM banks;
    x-tile stationary, W streamed as k-quads [128, 4*512].
  - LoRA: one K=128 matmul per (mt, oc) accumulates tT4.T @ a2rep into
    the same PSUM bank; a2rep holds 2*A replicated at the 4 stripes and
    zeros elsewhere, so the 4 partials fold in a single contraction.
Queues: W/consts on sync (HWDGE), out stores on scalar (HWDGE),
x loads alternating gpsimd/vector.
"""

import sys

if "/opt/trn_rl_repo" not in sys.path:
    sys.path.insert(0, "/opt/trn_rl_repo")

import numpy as np
import ml_dtypes

import concourse.bass as bass
import concourse.mybir as mybir
import concourse.tile as tile

N_CORES = 8
D = 4096
RANK = 16
ROWS_TOTAL = 4 * 4096          # 16384
M = ROWS_TOTAL // N_CORES      # 2048 rows per core
P = 128
KT = D // P                    # 32 k-tiles
OC = 512                       # o-chunk width (one PSUM bank)
N_OC = D // OC                 # 8
MT = M // P                    # 16 m-tiles
MQ = 4                         # m-tiles per quarter (PSUM banks)
NQ = MT // MQ                  # 4 quarters
KQ = 4                         # k-tiles per W quad DMA
N_KQ = KT // KQ                # 8

F32 = mybir.dt.float32
BF16 = mybir.dt.bfloat16
BF16_NP = ml_dtypes.bfloat16

N_WARMUP = 8


def split_wide_waits(nc, max_waits=1):
    """walrus in this container rejects >1 sync wait per instruction;
    move excess waits onto preceding same-engine NoOps."""
    n_split = 0
    for f in nc.m.functions:
        for bb in f.blocks:
            new_insts = []
            for inst in bb.instructions:
                si = getattr(inst, "sync_info", None)
                if si is not None and si.on_wait and len(si.on_wait) > max_waits:
                    waits = list(si.on_wait)
                    keep = waits[-max_waits:]
                    extra = waits[:-max_waits]
                    for i in range(0, len(extra), max_waits):
                        chunk = extra[i:i + max_waits]
                        nop = mybir.InstNoOp(
                            name=f"{inst.name}_wsplit{i}",
                            sync_info=mybir.SyncInfo(on_wait=chunk, on_update=[]),
                            bass_nofuse=True,
                            engine=inst.engine,
                        )
                        new_insts.append(nop)
                        n_split += 1
                    si.on_wait = keep
                new_insts.append(inst)
            bb.instructions[:] = new_insts
    return n_split


def build_program():
    nc = bass.Bass()
    xt = nc.declare_dram_parameter("xt", [D, M], BF16, isOutput=False)
    wt = nc.declare_dram_parameter("wt", [D, D], BF16, isOutput=False)
    # bmat pre-arranged on host: [128, KT*RANK], col-block k = rows k*128..+128
    bmat = nc.declare_dram_parameter("bmat", [P, KT * RANK], BF16, isOutput=False)
    # a2rep: 2*A replicated at partition stripes {32j..32j+15}, zeros elsewhere
    a2rep = nc.declare_dram_parameter("a2rep", [P, D], BF16, isOutput=False)
    out = nc.declare_dram_parameter("out", [M, D], F32, isOutput=True)

    with tile.TileContext(nc) as tc:
        with (
            tc.tile_pool(name="xpool", bufs=1) as xpool,
            tc.tile_pool(name="wpool", bufs=10) as wpool,
            tc.tile_pool(name="opool", bufs=3) as opool,
            tc.tile_pool(name="cpool", bufs=1) as cpool,
            tc.tile_pool(name="tpool", bufs=1) as tpool,
            tc.tile_pool(name="ppool", bufs=8, space="PSUM") as ppool,
        ):
            # constants on sync queue
            btile = cpool.tile([P, KT * RANK], BF16, tag="bt")
            nc.sync.dma_start(btile[:], bmat[:])
            atile = cpool.tile([P, D], BF16, tag="at")
            nc.sync.dma_start(atile[:], a2rep[:])

            # x fully resident: col block k holds x.T[k*128:(k+1)*128, :]
            xall = xpool.tile([P, KT * M], BF16, tag="x")

            def xsl(k, c0, cw):
                return xall[:, k * M + c0: k * M + c0 + cw]

            for k in range(KT):
                eng = nc.gpsimd if k % 2 == 0 else nc.scalar
                eng.dma_start(
                    xsl(k, 0, M),
                    xt[k * P:(k + 1) * P, :],
                )

            # tT4 [128, M]: stage-A partials at stripes {32j..32j+15},
            # memset clears the garbage stripes (they feed the K=128
            # folded accumulation; NaN garbage would poison psum).
            tT4 = tpool.tile([P, M], BF16, tag="tT")
            nc.vector.memset(tT4[:], 0.0)

            # HAM warmup: ~3.5us of dummy matmuls so the PE clock is at
            # 8/8 before real work lands.
            junk = ppool.tile([P, OC], F32, tag="acc", name="junk")
            for i in range(N_WARMUP):
                nc.tensor.matmul(
                    junk[:],
                    btile[:, :P],
                    btile[:, :OC],
                    start=(i == 0),
                    stop=(i == N_WARMUP - 1),
                )

            # stage A emission is interleaved with main oc0/q0 below so
            # the x-load phase keeps the PE fed (h0 + q0 = 5 MMs per
            # arriving k-tile).
            def stage_a(h):
                # tT4[32j+r, h*512+m] = partial_j[r, m]
                #   = sum_{k=j mod 4} B[k-tile].T @ x[k-tile, m-chunk h]
                pa = [
                    ppool.tile([P, OC], F32, tag="acc", name=f"pa_{h}_{j}")
                    for j in range(4)
                ]
                for k in range(KT):
                    j = k % 4
                    nc.tensor.matmul(
                        pa[j][32 * j:32 * j + RANK, :],
                        btile[:, k * RANK:(k + 1) * RANK],
                        xsl(k, h * OC, OC),
                        start=(k < 4),
                        stop=(k >= KT - 4),
                        tile_position=(0, 32 * j),
                    )
                for j in range(4):
                    nc.vector.tensor_copy(
                        tT4[32 * j:32 * j + RANK, h * OC:(h + 1) * OC],
                        pa[j][32 * j:32 * j + RANK, :],
                    )

            def main_quarter(oc, q, wtiles):
                psq = [
                    ppool.tile([P, OC], F32, tag="acc", name=f"ps_{oc}_{q}_{mi}")
                    for mi in range(MQ)
                ]
                for k4 in range(N_KQ):
                    for kk in range(KQ):
                        k = KQ * k4 + kk
                        for mi in range(MQ):
                            mt = q * MQ + mi
                            nc.tensor.matmul(
                                psq[mi][:],
                                xsl(k, mt * P, P),
                                wtiles[k4][:, kk * OC:(kk + 1) * OC],
                                start=(k == 0),
                                stop=False,
                            )
                # LoRA fold + copy-out + batched store (scalar queue)
                ot = opool.tile([P, MQ * OC], F32, tag="ot")
                for mi in range(MQ):
                    mt = q * MQ + mi
                    nc.tensor.matmul(
                        psq[mi][:],
                        tT4[:, mt * P:(mt + 1) * P],
                        atile[:, oc * OC:(oc + 1) * OC],
                        start=False,
                        stop=True,
                    )
                    nc.vector.tensor_copy(
                        ot[:, mi * OC:(mi + 1) * OC], psq[mi][:]
                    )
                nc.scalar.dma_start(
                    out[q * MQ * P:(q + 1) * MQ * P,
                        oc * OC:(oc + 1) * OC].rearrange(
                            "(b p) c -> p b c", p=P),
                    ot.rearrange("p (b c) -> p b c", b=MQ),
                )

            def w_load(oc):
                wtiles = []
                for k4 in range(N_KQ):
                    wtile = wpool.tile([P, KQ * OC], BF16, tag="wt")
                    nc.sync.dma_start(
                        wtile.rearrange("p (b c) -> p b c", b=KQ),
                        wt[k4 * KQ * P:(k4 + 1) * KQ * P,
                           oc * OC:(oc + 1) * OC].rearrange(
                               "(b p) c -> p b c", p=P),
                    )
                    wtiles.append(wtile)
                return wtiles

            # oc 0: stage A h0 first (4 banks) + quarter 0 (4 banks)
            # overlap the x-load; h1-3 follow once h0's banks free.
            wtiles = w_load(0)
            stage_a(0)
            main_quarter(0, 0, wtiles)
            for h in range(1, 4):
                stage_a(h)
            for q in range(1, NQ):
                main_quarter(0, q, wtiles)

            for oc in range(1, N_OC):
                wtiles = w_load(oc)
                for q in range(NQ):
                    main_quarter(oc, q, wtiles)

    split_wide_waits(nc)
    return nc


_NC_CACHE = [None]


def kernel(x, weight, lora_A, lora_B):
    from concourse.bass_utils import run_bass_kernel_spmd

    x = np.asarray(x, dtype=np.float32)
    weight = np.asarray(weight, dtype=np.float32)
    lora_A = np.asarray(lora_A, dtype=np.float32)
    lora_B = np.asarray(lora_B, dtype=np.float32)

    x2 = x.reshape(ROWS_TOTAL, D)
    wt = np.ascontiguousarray(weight.T).astype(BF16_NP)
    # a2rep: 2*A at stripes {32j..32j+15}, zeros elsewhere
    a2rep = np.zeros((P, D), dtype=BF16_NP)
    a2 = (2.0 * lora_A).astype(BF16_NP)
    for j in range(4):
        a2rep[32 * j:32 * j + RANK, :] = a2
    # pre-arrange B: [128, KT*RANK], col-block k holds rows k*128..(k+1)*128
    bmat = np.ascontiguousarray(
        lora_B.reshape(KT, P, RANK).transpose(1, 0, 2).reshape(P, KT * RANK)
    ).astype(BF16_NP)

    in_maps = []
    for c in range(N_CORES):
        xt_c = np.ascontiguousarray(
            x2[c * M:(c + 1) * M].T
        ).astype(BF16_NP)
        in_maps.append({"xt": xt_c, "wt": wt, "bmat": bmat, "a2rep": a2rep})

    if _NC_CACHE[0] is None:
        _NC_CACHE[0] = build_program()
    nc = _NC_CACHE[0]

    res = run_bass_kernel_spmd(nc, in_maps, list(range(N_CORES)))
    out = np.concatenate(
        [res.results[c]["out"] for c in range(N_CORES)], axis=0
    )
    return out.reshape(x.shape)


# revision 6
# speedup vs baseline: 1.0993x; 1.0351x over previous
"""LoRA layer kernel for Trainium2, 8-core data-parallel.

out = x @ W.T + 2.0 * ((x @ B) @ A)
  x: (4, 4096, 4096) f32, W: (4096, 4096), A: (16, 4096), B: (4096, 16)

Strategy: flatten x to (16384, 4096) rows, shard rows across 8 cores
(2048 rows each), replicate W/A/B. All matmul operands in bf16 (PSUM
accumulation stays fp32; end-to-end rel err ~2.4e-3 vs the 2e-2 gate).

Per core, single x-resident block (2048 rows = 128 KB/partition bf16):
  - stage A: tT = (x @ B).T computed 4-way column-tiled: k-tile k goes
    to PE col group k%4 (derived from out base partition 32j), partials
    land in 4 PSUM banks at partition stripes {32j..32j+15}; copies
    assemble tT4 [128, 2048] with zeros (memset) in the gap stripes.
  - main GEMM: per o-chunk (512 cols), 4 mt-quarters of 4 PSUM banks;
    x-tile stationary, W streamed as k-quads [128, 4*512].
  - LoRA: one K=128 matmul per (mt, oc) accumulates tT4.T @ a2rep into
    the same PSUM bank; a2rep holds 2*A replicated at the 4 stripes and
    zeros elsewhere, so the 4 partials fold in a single contraction.
Queues: W/consts on sync (HWDGE), out stores on scalar (HWDGE),
x loads alternating gpsimd/vector.
"""

import sys

if "/opt/trn_rl_repo" not in sys.path:
    sys.path.insert(0, "/opt/trn_rl_repo")

import numpy as np
import ml_dtypes

import concourse.bass as bass
import concourse.mybir as mybir
import concourse.tile as tile

N_CORES = 8
D = 4096
RANK = 16
ROWS_TOTAL = 4 * 4096          # 16384
M = ROWS_TOTAL // N_CORES      # 2048 rows per core
P = 128
KT = D // P                    # 32 k-tiles
OC = 512                       # o-chunk width (one PSUM bank)
N_OC = D // OC                 # 8
MT = M // P                    # 16 m-tiles
MQ = 4                         # m-tiles per quarter (PSUM banks)
NQ = MT // MQ                  # 4 quarters
KQ = 4                         # k-tiles per W quad DMA
N_KQ = KT // KQ                # 8

F32 = mybir.dt.float32
BF16 = mybir.dt.bfloat16
BF16_NP = ml_dtypes.bfloat16

N_WARMUP = 8


def split_wide_waits(nc, max_waits=1):
    """walrus in this container rejects >1 sync wait per instruction;
    move excess waits onto preceding same-engine NoOps."""
    n_split = 0
    for f in nc.m.functions:
        for bb in f.blocks:
            new_insts = []
            for inst in bb.instructions:
                si = getattr(inst, "sync_info", None)
                if si is not None and si.on_wait and len(si.on_wait) > max_waits:
                    waits = list(si.on_wait)
                    keep = waits[-max_waits:]
                    extra = waits[:-max_waits]
                    for i in range(0, len(extra), max_waits):
                        chunk = extra[i:i + max_waits]
                        nop = mybir.InstNoOp(
                            name=f"{inst.name}_wsplit{i}",
                            sync_info=mybir.SyncInfo(on_wait=chunk, on_update=[]),
                            bass_nofuse=True,
                            engine=inst.engine,
                        )
                        new_insts.append(nop)
                        n_split += 1
                    si.on_wait = keep
                new_insts.append(inst)
            bb.instructions[:] = new_insts
    return n_split


def build_program():
    nc = bass.Bass()
    xt = nc.declare_dram_parameter("xt", [D, M], BF16, isOutput=False)
    wt = nc.declare_dram_parameter("wt", [D, D], BF16, isOutput=False)
    # bmat pre-arranged on host: [128, KT*RANK], col-block k = rows k*128..+128
    bmat = nc.declare_dram_parameter("bmat", [P, KT * RANK], BF16, isOutput=False)
    # a2rep: 2*A replicated at partition stripes {32j..32j+15}, zeros elsewhere
    a2rep = nc.declare_dram_parameter("a2rep", [P, D], BF16, isOutput=False)
    out = nc.declare_dram_parameter("out", [M, D], F32, isOutput=True)

    with tile.TileContext(nc) as tc:
        with (
            tc.tile_pool(name="xpool", bufs=1) as xpool,
            tc.tile_pool(name="wpool", bufs=10) as wpool,
            tc.tile_pool(name="opool", bufs=3) as opool,
            tc.tile_pool(name="cpool", bufs=1) as cpool,
            tc.tile_pool(name="tpool", bufs=1) as tpool,
            tc.tile_pool(name="ppool", bufs=8, space="PSUM") as ppool,
        ):
            # constants on sync queue
            btile = cpool.tile([P, KT * RANK], BF16, tag="bt")
            nc.sync.dma_start(btile[:], bmat[:])
            atile = cpool.tile([P, D], BF16, tag="at")
            nc.sync.dma_start(atile[:], a2rep[:])

            # x fully resident: col block k holds x.T[k*128:(k+1)*128, :]
            xall = xpool.tile([P, KT * M], BF16, tag="x")

            def xsl(k, c0, cw):
                return xall[:, k * M + c0: k * M + c0 + cw]

            # m-split x loads: m-quarter mq unlocks mt-quarter mq and
            # stage-A chunk h=mq after only 1/4 of x has landed, keeping
            # the PE fed through the HBM-bound load phase.
            nx = 0
            for mq in range(NQ):
                for k in range(KT):
                    eng = nc.gpsimd if nx % 2 == 0 else nc.scalar
                    nx += 1
                    eng.dma_start(
                        xsl(k, mq * OC, OC),
                        xt[k * P:(k + 1) * P, mq * OC:(mq + 1) * OC],
                    )

            # tT4 [128, M]: stage-A partials at stripes {32j..32j+15},
            # memset clears the garbage stripes (they feed the K=128
            # folded accumulation; NaN garbage would poison psum).
            tT4 = tpool.tile([P, M], BF16, tag="tT")
            nc.vector.memset(tT4[:], 0.0)

            # HAM warmup: ~3.5us of dummy matmuls so the PE clock is at
            # 8/8 before real work lands.
            junk = ppool.tile([P, OC], F32, tag="acc", name="junk")
            for i in range(N_WARMUP):
                nc.tensor.matmul(
                    junk[:],
                    btile[:, :P],
                    btile[:, :OC],
                    start=(i == 0),
                    stop=(i == N_WARMUP - 1),
                )

            # stage A emission is interleaved with main oc0/q0 below so
            # the x-load phase keeps the PE fed (h0 + q0 = 5 MMs per
            # arriving k-tile).
            def stage_a(h):
                # tT4[32j+r, h*512+m] = partial_j[r, m]
                #   = sum_{k=j mod 4} B[k-tile].T @ x[k-tile, m-chunk h]
                pa = [
                    ppool.tile([P, OC], F32, tag="acc", name=f"pa_{h}_{j}")
                    for j in range(4)
                ]
                for k in range(KT):
                    j = k % 4
                    nc.tensor.matmul(
                        pa[j][32 * j:32 * j + RANK, :],
                        btile[:, k * RANK:(k + 1) * RANK],
                        xsl(k, h * OC, OC),
                        start=(k < 4),
                        stop=(k >= KT - 4),
                        tile_position=(0, 32 * j),
                    )
                for j in range(4):
                    nc.vector.tensor_copy(
                        tT4[32 * j:32 * j + RANK, h * OC:(h + 1) * OC],
                        pa[j][32 * j:32 * j + RANK, :],
                    )

            def main_quarter(oc, q, wtiles):
                psq = [
                    ppool.tile([P, OC], F32, tag="acc", name=f"ps_{oc}_{q}_{mi}")
                    for mi in range(MQ)
                ]
                for k4 in range(N_KQ):
                    for kk in range(KQ):
                        k = KQ * k4 + kk
                        for mi in range(MQ):
                            mt = q * MQ + mi
                            nc.tensor.matmul(
                                psq[mi][:],
                                xsl(k, mt * P, P),
                                wtiles[k4][:, kk * OC:(kk + 1) * OC],
                                start=(k == 0),
                                stop=False,
                            )
                # LoRA fold + copy-out + batched store (scalar queue)
                ot = opool.tile([P, MQ * OC], F32, tag="ot")
                for mi in range(MQ):
                    mt = q * MQ + mi
                    nc.tensor.matmul(
                        psq[mi][:],
                        tT4[:, mt * P:(mt + 1) * P],
                        atile[:, oc * OC:(oc + 1) * OC],
                        start=False,
                        stop=True,
                    )
                    nc.vector.tensor_copy(
                        ot[:, mi * OC:(mi + 1) * OC], psq[mi][:]
                    )
                seng = nc.scalar if (oc * NQ + q) % 2 == 0 else nc.sync
                seng.dma_start(
                    out[q * MQ * P:(q + 1) * MQ * P,
                        oc * OC:(oc + 1) * OC].rearrange(
                            "(b p) c -> p b c", p=P),
                    ot.rearrange("p (b c) -> p b c", b=MQ),
                )

            def w_load(oc):
                wtiles = []
                for k4 in range(N_KQ):
                    wtile = wpool.tile([P, KQ * OC], BF16, tag="wt")
                    nc.sync.dma_start(
                        wtile.rearrange("p (b c) -> p b c", b=KQ),
                        wt[k4 * KQ * P:(k4 + 1) * KQ * P,
                           oc * OC:(oc + 1) * OC].rearrange(
                               "(b p) c -> p b c", p=P),
                    )
                    wtiles.append(wtile)
                return wtiles

            # oc 0: stage A h0 first (4 banks) + quarter 0 (4 banks)
            # overlap the x-load; h1-3 follow once h0's banks free.
            wtiles = w_load(0)
            stage_a(0)
            main_quarter(0, 0, wtiles)
            for h in range(1, 4):
                stage_a(h)
            for q in range(1, NQ):
                main_quarter(0, q, wtiles)

            for oc in range(1, N_OC):
                wtiles = w_load(oc)
                for q in range(NQ):
                    main_quarter(oc, q, wtiles)

    split_wide_waits(nc)
    return nc


_NC_CACHE = [None]


def kernel(x, weight, lora_A, lora_B):
    from concourse.bass_utils import run_bass_kernel_spmd

    x = np.asarray(x, dtype=np.float32)
    weight = np.asarray(weight, dtype=np.float32)
    lora_A = np.asarray(lora_A, dtype=np.float32)
    lora_B = np.asarray(lora_B, dtype=np.float32)

    x2 = x.reshape(ROWS_TOTAL, D)
    wt = np.ascontiguousarray(weight.T).astype(BF16_NP)
    # a2rep: 2*A at stripes {32j..32j+15}, zeros elsewhere
    a2rep = np.zeros((P, D), dtype=BF16_NP)
    a2 = (2.0 * lora_A).astype(BF16_NP)
    for j in range(4):
        a2rep[32 * j:32 * j + RANK, :] = a2
    # pre-arrange B: [128, KT*RANK], col-block k holds rows k*128..(k+1)*128
    bmat = np.ascontiguousarray(
        lora_B.reshape(KT, P, RANK).transpose(1, 0, 2).reshape(P, KT * RANK)
    ).astype(BF16_NP)

    in_maps = []
    for c in range(N_CORES):
        xt_c = np.ascontiguousarray(
            x2[c * M:(c + 1) * M].T
        ).astype(BF16_NP)
        in_maps.append({"xt": xt_c, "wt": wt, "bmat": bmat, "a2rep": a2rep})

    if _NC_CACHE[0] is None:
        _NC_CACHE[0] = build_program()
    nc = _NC_CACHE[0]

    res = run_bass_kernel_spmd(nc, in_maps, list(range(N_CORES)))
    out = np.concatenate(
        [res.results[c]["out"] for c in range(N_CORES)], axis=0
    )
    return out.reshape(x.shape)
